# revision 57
# baseline (speedup 1.0000x reference)
"""TRN2 Bass kernel for nn_AttentionLayerDecoder (B=2, N=2048, HD=2048,
NH=16, KVH=4): RMSNorm -> GQA attention (inverted causal mask, no scaling)
-> output projection.

Sharding: 8 cores = (batch b in {0,1}) x (kv-group g in {0..3}).  Each core
computes 4 q-heads + its kv-head and a partial output projection
(contribution of its 512 columns of `a` through Wo); the host sums the 4
partials per batch.  All device tensors are pre-transposed on the host so
every matmul contraction sits on the partition axis; RMSNorm's norm_w is
folded into the weight matrices and the row scale s[n] (computed host-side,
shipped replicated across partitions) is applied to q/k/v out of PSUM.

Design notes (v5):
- Scores/Q/K stay f32r (attention is peaked; score rounding flips
  near-ties).  V / attention-weights / Wo / out ride bf16 (linear error
  only).
- HW (unlike the CoreSim cost model) pays ~47ns per stationary (ldweights)
  reload, measured via micro-bench; the kernel is organized around
  stationary reuse:
  * phase B runs jt-outer / head-inner: per j-tile the kT stationary is
    loaded twice (heads 0,1 then 2,3 around the AV block) and the v4
    stationary once for 4 back-to-back AV matmuls (vs 12 reloads/tile for
    the head-outer order).
  * phase C puts tokens on the out partition axis (out dram is [N, DOUT]):
    stationary is a 128-token a_st slice reused across two 512-wide wo
    matmuls into paired psum banks; one [128, 2048] DMA per token subtile.
- Softmax denominator: exp tiles accumulate on the DVE in bf16 (measured
  ~114ns/[128,512] tile on HW, 5x faster than f32) with copy-init, one
  ones-matmul per (head, i-chunk) collapses partitions; gpsimd (Pool) does
  NO elementwise work - its tensor ops measure 2.6x slower than the model.
  All 4 heads' exp tiles live in one [128, H, CH] tile (ex4) so full
  j-tiles accumulate with a single 4*CH-wide DVE add instead of 4 chained
  ones; diagonal tiles fall back to per-head capped adds.
- Inverted-causal mask via a single [128,128] strictly-lower-tri bf16
  const applied to the 127-wide mixed stripe of diagonal tiles only.
- RMS scale s is folded into tok on the host (tok = (x*s)^T), so q/k/v
  leave PSUM as plain DVE copies.
- Phases are generators merged with a weighted (Bresenham) round-robin at
  ~1.5us quanta: A3 A2 (B3|A1) (B2|C3) (B1|A0) (B0|C2) C1 C0 - the
  in-order PE queue then always has projection/out-proj matmuls between B
  quanta that wait on Act's exp, and the shorter partner is spread across
  the longer phase's whole span (incl. B's denominator/normalize tail);
  within B, scores run LAG=2 tiles ahead of AV.
- Tile pools are hoisted out of the per-rep body (make_pools) so K-repeat
  timing builds measure steady-state iteration time; identical at reps=1.
- Row i=N-1 is fully masked -> uniform attention; its row is patched on
  the host (mean_j v = Wv @ mean(x*s)) and zeroed on device.
- out is written bf16 [N, DOUT]; host upcasts, sums the 4 kv-group
  partials per batch and adds bo.
"""
import numpy as np
from contextlib import ExitStack

import concourse.bass as bass
import concourse.tile as tile
from concourse import bacc, mybir
from concourse.bass_utils import run_bass_kernel_spmd

F32 = mybir.dt.float32
F32R = mybir.dt.float32r
BF16 = mybir.dt.bfloat16
BF16_NP = mybir.dt.np(mybir.dt.bfloat16)
AF = mybir.ActivationFunctionType
EPS = float(np.finfo(np.float32).eps)

B, N, HD = 2, 2048, 2048
NH, KVH = 16, 4
DD = HD // NH            # 128 head dim
H = NH // KVH            # 4 q-heads per kv-group / core
D = HD                   # model (contraction) dim
DOUT = HD
CH = 512                 # n/i chunk width (one PSUM bank at fp32)
N_CORES = 8
DT = D // 128            # 16 contraction tiles
NCH = N // CH            # 4 chunks
JT = N // 128            # 16 j tiles
JPC = CH // 128          # 4 j tiles per chunk
OT = DOUT // 128         # 16 output tiles
NMASK = CH // 128        # 4 diagonal mask variants
LAG = 3                  # B-phase software-pipeline depth


def make_pools(ctx, tc):
    """Tile pools are hoisted out of the per-rep kernel body so a K-repeat
    timing build measures steady-state iteration time (no per-rep pool
    teardown barriers); at reps=1 the graph is identical either way."""
    return dict(
        cpool=ctx.enter_context(tc.tile_pool(name="consts", bufs=1)),
        wpool=ctx.enter_context(tc.tile_pool(name="weights", bufs=1)),
        big=ctx.enter_context(tc.tile_pool(name="big", bufs=1)),
        tokp=ctx.enter_context(tc.tile_pool(name="tok", bufs=DT + 4)),
        vtp=ctx.enter_context(tc.tile_pool(name="vt", bufs=2)),
        stp=ctx.enter_context(tc.tile_pool(name="st", bufs=2)),
        expp=ctx.enter_context(
            tc.tile_pool(name="expp", bufs=LAG + 2)),
        obp=ctx.enter_context(tc.tile_pool(name="obp", bufs=2)),
        accp=ctx.enter_context(tc.tile_pool(name="accp", bufs=1)),
        psb=ctx.enter_context(tc.tile_pool(name="psb", bufs=1, space="PSUM")),
    )


def _attention_kernel(tc, ext, pools, state=None, first=True, last=True):
    nc = tc.nc
    if state is None:
        state = {}

    cpool = pools["cpool"]
    wpool = pools["wpool"]
    big = pools["big"]
    tokp = pools["tokp"]
    vtp = pools["vtp"]
    stp = pools["stp"]
    expp = pools["expp"]
    obp = pools["obp"]
    accp = pools["accp"]
    psb = pools["psb"]

    # ---- consts ----
    ones_b = cpool.tile([128, 128], BF16, tag="ones_b")
    nc.gpsimd.memset(ones_b[:], 1.0)
    # ---- weights: wq on the Act queue, one strided DMA per head so the
    # first q-chain starts after a quarter of the bytes; wk/wv go on SP
    # inside A(3) ----
    wq_e = []
    for e in range(H):
        w = wpool.tile([128, DT * 128], F32R, name=f"wqe{e}", tag=f"wqe{e}")
        # head 0 rides the gpsimd queue: the Act queue opens with ~4us of
        # activation-table loads, which would delay the very first q-chain.
        eng = nc.gpsimd if e == 0 else nc.scalar
        eng.dma_start(
            w[:].rearrange("p (a c) -> p a c", a=DT),
            ext["wq"][:, e * 128:(e + 1) * 128].rearrange(
                "(a p) c -> p a c", p=128))
        wq_e.append(w)
    identr = cpool.tile([128, 128], F32, tag="identr")
    nc.gpsimd.dma_start(identr[:], ext["identr"][:])
    # single [128,128] strictly-lower-triangular mask: a diagonal j-tile at
    # depth d only has mixed columns in the 127-wide stripe starting at
    # column 128*d (earlier columns are fully allowed, later fully masked).
    tri = cpool.tile([128, 128], BF16, tag="tri")
    nc.gpsimd.dma_start(tri[:], ext["tri"][:])

    # wk/wv land in one strided DMA each ([2048,128] -> [128, 16*128]);
    # per-tile DMAs would each pay the 500ns descriptor-gen floor.
    wk_all = wpool.tile([128, DT * DD], F32R, tag="wk_all")
    wv_all = wpool.tile([128, DT * DD], F32R, tag="wv_all")
    wk_t = [wk_all[:, dt * DD:(dt + 1) * DD] for dt in range(DT)]
    wv_t = [wv_all[:, dt * DD:(dt + 1) * DD] for dt in range(DT)]
    wo_t = [wpool.tile([128, DOUT], BF16, name=f"wo{ht}", tag=f"wo{ht}")
            for ht in range(H)]

    qT = [big.tile([128, N], F32R, name=f"qT{e}", tag=f"qT{e}")
          for e in range(H)]
    kT = big.tile([128, N], F32R, tag="kT")
    # V^T per chunk, [128 j, 4*128 d].  NOTE: dma_start_transpose is NOT
    # dependency-tracked by the tile framework (races on HW); V transposes
    # go through the PE (identity matmul) instead.
    v4 = [big.tile([128, CH], BF16, name=f"v4_{c}", tag=f"v4_{c}")
          for c in range(NCH)]
    a_st = [big.tile([128, N], BF16, name=f"a{h}", tag=f"a{h}")
            for h in range(H)]

    # persists across reps in K-repeat builds so a rep's tail can prefetch
    # the next rep's boundary token chunks (at reps=1 nothing changes)
    tok_tiles = state.setdefault("tok_tiles", {})

    def tok_load(c):
        csl = slice(c * CH, (c + 1) * CH)
        tiles = []
        for dt in range(DT):
            t = tokp.tile([128, CH], F32R, tag="tok", name=f"tok{c}_{dt}")
            eng = nc.sync if dt % 2 == 0 else nc.gpsimd
            eng.dma_start(t[:], ext["tok"][dt * 128:(dt + 1) * 128, csl])
            tiles.append(t)
        tok_tiles[c] = tiles

    def phase_a(c, load_kv=False):
        """Generator: Q/K/V projection chains for one 512-token chunk,
        yielding at half-chain granularity so phase-B quanta interleave."""
        csl = slice(c * CH, (c + 1) * CH)
        tok_c = tok_tiles.pop(c)
        if load_kv:
            nc.sync.dma_start(
                wk_all[:].rearrange("p (a c) -> p a c", a=DT),
                ext["wk"].rearrange("(a p) c -> p a c", p=128))
            nc.sync.dma_start(
                wv_all[:].rearrange("p (a c) -> p a c", a=DT),
                ext["wv"].rearrange("(a p) c -> p a c", p=128))

        # RMS scale s is folded into tok on the host, so psum->sbuf moves are
        # plain copies.
        for e in range(H):
            ps_q = psb.tile([128, CH], F32, tag="mm", bufs=2)
            for dt in range(DT):
                nc.tensor.matmul(ps_q[:],
                                 wq_e[e][:, dt * 128:(dt + 1) * 128],
                                 tok_c[dt][:],
                                 start=(dt == 0), stop=(dt == DT - 1))
                if dt == DT // 2 - 1:
                    yield
            nc.vector.tensor_copy(qT[e][:, csl], ps_q[:])
            yield
        ps_k = psb.tile([128, CH], F32, tag="mm", bufs=2)
        for dt in range(DT):
            nc.tensor.matmul(ps_k[:], wk_t[dt][:], tok_c[dt][:],
                             start=(dt == 0), stop=(dt == DT - 1))
            if dt == DT // 2 - 1:
                yield
        nc.vector.tensor_copy(kT[:, csl], ps_k[:])
        yield
        ps_v = psb.tile([128, CH], F32, tag="mm", bufs=2)
        for dt in range(DT):
            nc.tensor.matmul(ps_v[:], wv_t[dt][:], tok_c[dt][:],
                             start=(dt == 0), stop=(dt == DT - 1))
            if dt == DT // 2 - 1:
                yield
        vt = vtp.tile([128, CH], F32, tag="vt", bufs=2)
        nc.vector.tensor_copy(vt[:], ps_v[:])
        ps_t = psb.tile([128, CH], F32, tag="mm", bufs=2)
        for js in range(JPC):
            nc.tensor.transpose(ps_t[:, js * 128:(js + 1) * 128],
                                vt[:, js * 128:(js + 1) * 128], identr[:])
        nc.scalar.copy(v4[c][:], ps_t[:])
        yield

    def phase_b(ic):
        """Generator.  jt-outer / 4-head-inner so the kT[jt] stationary is
        loaded twice per tile (h0,h1 then h2,h3) and the v4[jt] stationary
        once (4 AV matmuls back-to-back) -- stationary reloads are the
        dominant unmodeled HW cost.  The softmax count rides the DVE as a
        bf16 accumulate chain per head (copy-init), collapsed by one
        ones-matmul per (h, ic)."""
        isl = slice(ic * CH, (ic + 1) * CH)
        # descending j so the chain starts on a full tile (start=True must
        # cover the whole free range); diagonal partials come last with
        # their exp/mask/AV narrowed to the live column range.
        jts = [jt for jt in range(JT - 1, -1, -1) if 128 * jt + 127 > CH * ic]
        nst = len(jts)

        def cap_of(jt):
            t_off = CH * ic - 128 * jt
            if -CH < t_off < 127:
                d = -t_off // 128
                return min(CH, 128 * d + 127), d
            return CH, None

        ps_av = [psb.tile([128, CH], F32, tag=f"av{h}", bufs=1,
                          name=f"psav{h}_{ic}") for h in range(H)]
        # all 4 heads' exp-sums in one contiguous [128, H, CH] bf16 tile so
        # full j-tiles accumulate with ONE wide DVE add instead of 4 chained
        # ones (keeps the exp->AV->count chain off the critical path)
        acc = accp.tile([128, H, CH], BF16, tag="acc", bufs=1,
                        name=f"acc_{ic}")
        pend = {}

        def sc_one(h, ex4, jt, cap, d, scw):
            ps_sc = psb.tile([128, CH], F32, tag="sc", bufs=2, name="pssc")
            nc.tensor.matmul(ps_sc[:, :scw],
                             kT[:, jt * 128:(jt + 1) * 128],
                             qT[h][:, isl.start:isl.start + scw],
                             start=True, stop=True)
            nc.scalar.activation(ex4[:, h, :cap], ps_sc[:, :cap], AF.Exp)
            if d is not None:
                lo = 128 * d
                nc.vector.tensor_mul(ex4[:, h, lo:cap], ex4[:, h, lo:cap],
                                     tri[:, :cap - lo])

        def av_block(j2):
            jt2 = jts[j2]
            ex4, cap2 = pend.pop(j2)
            first, last = (j2 == 0), (j2 == nst - 1)
            vsl = v4[jt2 // JPC][:, (jt2 % JPC) * 128:(jt2 % JPC) * 128 + 128]
            for h in range(H):
                nc.tensor.matmul(ps_av[h][:, :cap2], vsl, ex4[:, h, :cap2],
                                 start=first, stop=last)
            if cap2 == CH:
                if first:
                    nc.vector.tensor_copy(acc[:], ex4[:])
                else:
                    nc.vector.tensor_add(acc[:], acc[:], ex4[:])
            else:
                for h in range(H):
                    if first:
                        nc.vector.tensor_copy(acc[:, h, :cap2],
                                              ex4[:, h, :cap2])
                        # zero the tail so the denominator matmul never
                        # reads uninitialized SBUF (only ic=3)
                        nc.vector.memset(acc[:, h, cap2:CH], 0.0)
                    else:
                        nc.vector.tensor_add(acc[:, h, :cap2],
                                             acc[:, h, :cap2],
                                             ex4[:, h, :cap2])

        for idx in range(nst + LAG):
            if idx < nst:
                jt = jts[idx]
                cap, d = cap_of(jt)
                # f32r matmul is full-rate only at free>=256; round the
                # scores width up to a 128 multiple >=256.
                scw = min(CH, max(256, ((cap + 127) // 128) * 128))
                ex4 = expp.tile([128, H, CH], BF16, tag="ex4", name="ex4")
                sc_one(0, ex4, jt, cap, d, scw)
                sc_one(1, ex4, jt, cap, d, scw)
                if idx >= LAG:
                    av_block(idx - LAG)
                sc_one(2, ex4, jt, cap, d, scw)
                sc_one(3, ex4, jt, cap, d, scw)
                pend[idx] = (ex4, cap)
            else:
                av_block(idx - LAG)
            yield
        for h in range(H):
            ps_cs = psb.tile([128, CH], F32, tag="sc", bufs=2, name="pscs")
            nc.tensor.matmul(ps_cs[:], ones_b[:], acc[:, h, :],
                             start=True, stop=True)
            if ic == NCH - 1:
                # column i=N-1 has no allowed j (and no tile writes it):
                # force denom 1 / sum 0 so the normalize writes 0 (the host
                # patches the real value).
                nc.vector.memset(ps_cs[:, CH - 1:CH], 1.0)
                nc.vector.memset(ps_av[h][:, CH - 1:CH], 0.0)
            rec = stp.tile([128, CH], F32, tag="rec", bufs=2)
            with nc.allow_low_precision(reason="softmax denom recip"):
                nc.vector.reciprocal(rec[:], ps_cs[:])
            nc.vector.tensor_mul(a_st[h][:, isl], ps_av[h][:], rec[:])
            yield

    def phase_c(c, load_wo=False):
        # out-proj with tokens on the output partition axis: stationary is a
        # 128-token slice of a_st (reused across two 512-wide wo matmuls), so
        # each stationary load serves 2 matmuls on HW (ldweights are the
        # dominant unmodeled HW cost).  out dram is [N, DOUT]; the host skips
        # its transpose.
        if load_wo:
            for ht in range(H):
                nc.sync.dma_start(wo_t[ht][:], ext["wo"][ht * 128:(ht + 1) * 128, :])
        for t in range(JPC):
            tsl = slice(c * CH + t * 128, c * CH + t * 128 + 128)
            ob = obp.tile([128, DOUT], BF16, tag="ob", bufs=2)
            for op2 in range(2):
                ps0 = psb.tile([128, CH], F32, tag="mm", bufs=2)
                ps1 = psb.tile([128, CH], F32, tag="mm", bufs=2)
                o0 = (2 * op2) * CH
                o1 = (2 * op2 + 1) * CH
                for ht in range(H):
                    nc.tensor.matmul(ps0[:], a_st[ht][:, tsl],
                                     wo_t[ht][:, o0:o0 + CH],
                                     start=(ht == 0), stop=(ht == H - 1))
                    nc.tensor.matmul(ps1[:], a_st[ht][:, tsl],
                                     wo_t[ht][:, o1:o1 + CH],
                                     start=(ht == 0), stop=(ht == H - 1))
                nc.vector.tensor_copy(ob[:, o0:o0 + CH], ps0[:])
                nc.vector.tensor_copy(ob[:, o1:o1 + CH], ps1[:])
                yield
            nc.sync.dma_start(ext["out"][tsl, :], ob[:])

    def run(*gens_w):
        """Weighted (Bresenham) merge of phase generators: each arg is
        (generator, expected_quanta).  Quanta are emitted so all generators
        exhaust together -- a strict 1:1 round-robin would drain the shorter
        partner early and leave the longer phase's tail (e.g. B's Act-bound
        AV flush + denominator/normalize chains) exposed on the in-order PE
        queue with nothing to interleave."""
        state = [[g, max(1, n), 0.0] for g, n in gens_w]
        total = max(n for _, n, _ in state)
        alive = list(state)
        while alive:
            for ent in list(alive):
                ent[2] += ent[1] / total
                while ent[2] >= 1.0 and ent in alive:
                    ent[2] -= 1.0
                    try:
                        next(ent[0])
                    except StopIteration:
                        alive.remove(ent)

    # quanta counts: A = 12, B(ic) = (16-4*ic) + LAG + 4, C = 8
    def nb(ic):
        return 16 - 4 * ic + LAG + 4

    if first:
        tok_load(3)
        tok_load(2)
    run((phase_a(3, load_kv=True), 12))
    tok_load(1)
    run((phase_a(2), 12))
    run((phase_b(3), nb(3)), (phase_a(1), 12))
    tok_load(0)
    run((phase_b(2), nb(2)), (phase_c(3, load_wo=True), 8))
    run((phase_b(1), nb(1)), (phase_a(0), 12))
    if not last:
        # prefetch the next rep's boundary chunks under the out-proj tail
        tok_load(3)
    # C1 joins the B0 merge so its a_st stationary loads are emitted BEFORE
    # B0's tail normalizes -- emitted after, the whole-tile dependency
    # tracking would stall C1's first matmuls ~4.7us behind all four heads'
    # denominator/reciprocal/normalize chains (seen in the reps=2 trace).
    run((phase_b(0), nb(0)), (phase_c(2), 8), (phase_c(1), 8))
    if not last:
        tok_load(2)
    run((phase_c(0), 8))
    return qT, kT, v4, a_st


def build_bass(reps=1):
    nc = bacc.Bacc("TRN2", target_bir_lowering=False, debug=False,
                   num_devices=N_CORES)
    ND = H * DD
    ext = {}
    ext["tok"] = nc.dram_tensor("tok", [D, N], F32R, kind="ExternalInput").ap()
    ext["wq"] = nc.dram_tensor("wq", [D, ND], F32R, kind="ExternalInput").ap()
    ext["wk"] = nc.dram_tensor("wk", [D, DD], F32R, kind="ExternalInput").ap()
    ext["wv"] = nc.dram_tensor("wv", [D, DD], F32R, kind="ExternalInput").ap()
    ext["wo"] = nc.dram_tensor("wo", [ND, DOUT], BF16, kind="ExternalInput").ap()
    ext["tri"] = nc.dram_tensor("tri", [128, 128], BF16,
                                kind="ExternalInput").ap()
    ext["identr"] = nc.dram_tensor("identr", [128, 128], F32,
                                   kind="ExternalInput").ap()
    ext["out"] = nc.dram_tensor("out", [N, DOUT], BF16, kind="ExternalOutput").ap()
    with tile.TileContext(nc) as tc:
        with ExitStack() as ctx:
            pools = make_pools(ctx, tc)
            state = {}
            for r in range(reps):
                _attention_kernel(tc, ext, pools, state,
                                  first=(r == 0), last=(r == reps - 1))
    nc.compile()
    return nc


def _make_tri():
    """Strictly-lower-triangular multiplicative mask: tri[p, t] = 1 iff
    p > t (keep j > i within a diagonal 128x128 block)."""
    p = np.arange(128)[:, None]
    t = np.arange(128)[None, :]
    return (p > t).astype(np.float32).astype(BF16_NP)


def _rms_scale(tokens_b):
    ms = np.mean(tokens_b.astype(np.float32) ** 2, axis=-1) + EPS
    return (1.0 / np.sqrt(ms)).astype(np.float32)    # [N]


def make_in_maps(tokens, norm_w, Wq, Wk, Wv, Wo):
    """Per-core input dict list (core = b*KVH + g).  The RMS row scale s is
    folded into tok host-side (tok = (x * s).T), so the device applies no
    per-token scaling."""
    tri = _make_tri()
    tok_b = [np.ascontiguousarray(
        (tokens[b] * _rms_scale(tokens[b])[:, None]).T)
        for b in range(B)]
    in_maps = []
    for core in range(N_CORES):
        b, g = divmod(core, KVH)
        # reference GQA: q-head h attends with kv-head h % KVH, so kv-group
        # g serves the interleaved q-heads {g, g+KVH, g+2*KVH, g+3*KVH}
        hidx = np.concatenate(
            [np.arange((g + KVH * j) * DD, (g + KVH * j + 1) * DD)
             for j in range(H)])
        in_maps.append({
            "tok": tok_b[b],
            "wq": np.ascontiguousarray((Wq[hidx] * norm_w[None, :]).T),
            "wk": np.ascontiguousarray(
                (Wk[g * DD:(g + 1) * DD] * norm_w[None, :]).T),
            "wv": np.ascontiguousarray(
                (Wv[g * DD:(g + 1) * DD] * norm_w[None, :]).T),
            "wo": np.ascontiguousarray(Wo[:, hidx].T).astype(BF16_NP),
            "tri": tri,
            "identr": np.eye(128, dtype=np.float32),
        })
    return in_maps


def assemble_out(core_outs, tokens, norm_w, Wv, bv, Wo, bo):
    """Sum per-core bf16 partials ([N, DOUT] layout), add bo, and patch the
    fully-masked last row (uniform attention = mean_j v)."""
    out = np.zeros((B, N, HD), np.float32)
    for b in range(B):
        acc = np.zeros((N, DOUT), np.float32)
        for g in range(KVH):
            acc += np.asarray(core_outs[b * KVH + g]).astype(np.float32)
        ob = acc + bo[None, :]
        # host patch for row i=N-1: attention is uniform over all j
        s = _rms_scale(tokens[b])
        xbar = (tokens[b] * s[:, None] * norm_w[None, :]).mean(axis=0)
        vbar = xbar @ Wv.T + bv          # [KVD]
        a_last = np.tile(vbar, H)        # head h uses kv-head h % KVH
        ob[N - 1, :] = a_last @ Wo.T + bo
        out[b] = ob
    return out


_NC_CACHE = {}


def _get_nc():
    if "nc" not in _NC_CACHE:
        _NC_CACHE["nc"] = build_bass()
    return _NC_CACHE["nc"]


def _kernel_numpy(tokens, norm_w, Wq, bq, Wk, bk, Wv, bv, Wo, bo):
    """Reference-exact numpy fallback (used only if biases are nonzero,
    which the benchmark inputs never are)."""
    tokens = np.asarray(tokens, np.float32)
    x = tokens * (1.0 / np.sqrt((tokens ** 2).mean(-1, keepdims=True) + EPS))
    x = x * np.asarray(norm_w)[None, None, :]
    q = (x @ np.asarray(Wq).T + bq).reshape(B, N, NH, DD).transpose(0, 2, 1, 3)
    k = (x @ np.asarray(Wk).T + bk).reshape(B, N, KVH, DD).transpose(0, 2, 1, 3)
    v = (x @ np.asarray(Wv).T + bv).reshape(B, N, KVH, DD).transpose(0, 2, 1, 3)
    k = np.tile(k, (1, NH // KVH, 1, 1))
    v = np.tile(v, (1, NH // KVH, 1, 1))
    i = np.arange(N)
    mask = i[None, :] <= i[:, None]
    out = np.zeros((B, N, HD), np.float32)
    for b in range(B):
        for h in range(NH):
            sc = q[b, h] @ k[b, h].T
            sc = np.where(mask, np.float32(-1e9), sc)
            m = sc.max(1, keepdims=True)
            e = np.exp(sc - m)
            a = (e / e.sum(1, keepdims=True)) @ v[b, h]
            out[b, :, h * DD:(h + 1) * DD] = a
    return (out.reshape(B * N, HD) @ np.asarray(Wo).T + bo).reshape(B, N, HD)


def kernel(tokens, norm_w, Wq, bq, Wk, bk, Wv, bv, Wo, bo):
    tokens = np.asarray(tokens, np.float32)
    norm_w = np.asarray(norm_w, np.float32)
    Wq, Wk, Wv, Wo = (np.asarray(a, np.float32) for a in (Wq, Wk, Wv, Wo))
    bq, bk, bv, bo = (np.asarray(a, np.float32) for a in (bq, bk, bv, bo))
    if any(np.abs(b).max() > 0 for b in (bq, bk, bv)):
        # the device kernel folds norm into the weights, which only admits
        # zero q/k/v biases (benchmark inputs are zero-filled).
        return _kernel_numpy(tokens, norm_w, Wq, bq, Wk, bk, Wv, bv, Wo, bo)

    nc = _get_nc()
    in_maps = make_in_maps(tokens, norm_w, Wq, Wk, Wv, Wo)
    res = run_bass_kernel_spmd(nc, in_maps, core_ids=list(range(N_CORES)))
    out = assemble_out([r["out"] for r in res.results],
                       tokens, norm_w, Wv, bv, Wo, bo)
    if not np.isfinite(out).all():
        # transient dispatch corruption (axon tunnel glitch / cold-start
        # device state): retry once
        res = run_bass_kernel_spmd(nc, in_maps, core_ids=list(range(N_CORES)))
        out = assemble_out([r["out"] for r in res.results],
                           tokens, norm_w, Wv, bv, Wo, bo)
    return out

